# revision 4
# baseline (speedup 1.0000x reference)
"""DeepseekV2 MLA (chunked-softmax MQA) on 8 trn2 NeuronCores.

Sharding: tensor-parallel over heads (16 heads / 8 cores = 2 heads per core);
the 576-wide latent KV cache is replicated per core. Each core computes its two
heads' full attention output [1024, 256]; the host concatenates along the
feature axis. All matmuls run in bf16 with fp32 PSUM accumulation.

Instead of the weight-absorbed form (score K=576, PV over the 512-wide
latent), each core un-absorbs the projections for its two heads up front:
  kT_h = w_kc[h] @ kv_loraT                 (PE, [128 nope, S])
  v    = kv_loraT.T @ [w_vc[h0]|w_vc[h1]]   ([128 s, 256] tiles)
so the inner loops shrink to
  scoresT = kT_h.T @ q_nopeT + ropeT.T @ q_peT   (K=128 + K=64 paired)
  pT      = exp(scale * scoresT)                  (ACT, PSUM->SBUF bf16)
  attnT  += v_tile.T @ pT                         (PE accum, [128 v, 512 t])
  out     = attnT.T via PE transpose, * 1/denom, DMA out.

Pipeline balancing (phase B is ACT/DVE-gated otherwise):
  - score PSUM tiles are allocated as [128, 1024] pairs so one ACT exp
    covers two s-tiles (amortizes ACT instruction overhead)
  - softmax denominator partial sums alternate DVE / GpSimd
  - PV matmuls lag two s-tile groups behind so exp latency never stalls PE
  - k-gen (head 0) + v-gen interleave with the first phase's groups, and
    k-gen (head 1) with the second phase's, filling PE while ACT is the
    per-group rate limiter
  - each phase's epilogue (denominator transpose, output transposes, DMA)
    is deferred into the next phase's early groups
"""

import os
import sys

import numpy as np
import ml_dtypes

for _p in ("/opt/trn_rl_repo",):
    if os.path.isdir(_p) and _p not in sys.path:
        sys.path.append(_p)

import concourse.bass as bass
import concourse.mybir as mybir
import concourse.tile as tile
from concourse.bass_utils import run_bass_kernel_spmd
from concourse.masks import make_identity
from concourse.vector_clock import ScopedClock, VectorClock

# ---------------------------------------------------------------- constants
NOPE, ROPE, LORA, VDIM = 128, 64, 512, 128
T, H, S = 1024, 16, 8192
D = LORA + ROPE            # 576 latent dim
SCALING = (NOPE + ROPE) ** -0.5
N_CORES = 8
HPC = H // N_CORES         # heads per core
NST = S // 128             # 64 s-tiles
NTB = T // 512             # 2 t-blocks
BF16 = mybir.dt.bfloat16
FP32 = mybir.dt.float32
NPBF = ml_dtypes.bfloat16


# ------------------------------------------------- walrus drain workaround
def _patch_tile_drain():
    """The neuronxcc walrus in this container rejects DRAIN instructions
    carrying more than ~2 sync waits ("Too many sync wait commands").
    Split the TileContext exit drain into one drain per processor tick;
    the waits execute sequentially on SP before the all-engine barrier,
    preserving the original semantics."""
    if getattr(tile.TileContext, "_drain_split_patched", False):
        return

    def _drain_and_barrier_split(self, tick_clock, wait_clock):
        gcv = tick_clock.global_clock
        n = len(gcv)
        for proc in range(n):
            t = gcv[proc]
            if t <= 0:
                continue
            vc = VectorClock([0] * n)
            vc.require_at_least(proc, t)
            d = self.nc.sync.drain()
            wait_clock.add_sem_waits(d.ins, ScopedClock({None: vc}))
        self.nc.all_engine_barrier()
        assert self.sems is not None
        popped = self.nc._tile_sem_poison_stack.pop()
        assert popped is self._sem_poison
        self.nc.clear_and_free_semaphores(list(self.sems.allocated().values()))
        self.nc.all_engine_barrier()

    tile.TileContext._drain_and_barrier = _drain_and_barrier_split

    # Same walrus limitation for regular instructions: peel all but the last
    # sync wait off onto same-engine NOPs inserted immediately before the
    # instruction. The engine executes its queue in order, so waiting on the
    # NOPs first is equivalent to one multi-wait instruction.
    orig_add = tile.TileContext._add_instruction

    def _add_instruction_split_waits(self, inst):
        si = inst.sync_info
        if si is not None:
            waits = si.on_wait
            if waits and len(waits) > 1:
                for w in waits[:-1]:
                    nop = mybir.InstNoOp(
                        name=self.nc.get_next_instruction_name(), ins=[], outs=[]
                    )
                    nop.engine = inst.engine
                    nop.sync_info = mybir.SyncInfo(on_wait=[w], on_update=[])
                    orig_add(self, nop)
                inst.sync_info = mybir.SyncInfo(
                    on_wait=[waits[-1]], on_update=si.on_update
                )
        orig_add(self, inst)

    tile.TileContext._add_instruction = _add_instruction_split_waits
    tile.TileContext._drain_split_patched = True


# ------------------------------------------------------------ bass program
MM_KINDS = {}


def _build_program():
    _patch_tile_drain()
    nc = bass.Bass()
    _orig_mm = nc.tensor.matmul

    def _mm_logged(out, lhsT, rhs, kind="?", **kw):
        inst = _orig_mm(out, lhsT, rhs, **kw)
        MM_KINDS[inst.ins.name] = kind
        return inst

    nc.tensor.matmul = _mm_logged
    qnT = nc.declare_dram_parameter("qnT", [HPC, NOPE, T], BF16, isOutput=False)
    qpT = nc.declare_dram_parameter("qpT", [HPC, ROPE, T], BF16, isOutput=False)
    kvT = nc.declare_dram_parameter("kvT", [LORA, S], BF16, isOutput=False)
    kvr = nc.declare_dram_parameter("kvr", [128, S], BF16, isOutput=False)
    wkcT = nc.declare_dram_parameter("wkcT", [HPC, 4, 128, NOPE], BF16, isOutput=False)
    wv2 = nc.declare_dram_parameter("wv2", [4, 128, HPC * VDIM], BF16, isOutput=False)
    out = nc.declare_dram_parameter("out", [T, HPC * VDIM], FP32, isOutput=True)

    Exp = mybir.ActivationFunctionType.Exp

    with tile.TileContext(nc) as tc:
        with (
            tc.tile_pool(name="res", bufs=1) as res,
            tc.tile_pool(name="ptp", bufs=6) as ptp,
            tc.tile_pool(name="attnsb", bufs=2) as attnsb,
            tc.tile_pool(name="smsb", bufs=4) as smsb,
            tc.tile_pool(name="outsb", bufs=3) as outsb,
            tc.tile_pool(name="ps_sc", bufs=3, space="PSUM") as ps_sc,
            tc.tile_pool(name="ps_attn", bufs=1, space="PSUM") as ps_attn,
            tc.tile_pool(name="ps_out", bufs=1, space="PSUM") as ps_out,
        ):
            # PE warmup first: ~5us of matmuls on local data so HAM
            # un-throttles and the first real matmuls run at full clock
            warm = res.tile([128, 512], BF16, tag="warm")
            nc.vector.memset(warm[:], 0.0)
            wu_ps = ps_sc.tile([128, 1024], FP32, tag="sc", name="wu_ps")
            for _w in range(20):
                nc.tensor.matmul(wu_ps[:, 0:512], warm[:, 0:128], warm[:],
                                 kind="warm")

            # ---------------- resident loads
            qnT_sb = res.tile([NOPE, HPC * T], BF16, tag="qnt")
            qpT_sb = res.tile([128, HPC * T], BF16, tag="qpt")
            for h in range(HPC):
                nc.scalar.dma_start(qnT_sb[:, h * T:(h + 1) * T], qnT[h])
                # rope rows duplicated to partitions 64:128 so pairs of K=64
                # rope matmuls run concurrently in disjoint PE row-groups
                nc.scalar.dma_start(qpT_sb[0:ROPE, h * T:(h + 1) * T], qpT[h])
                nc.scalar.dma_start(qpT_sb[ROPE:128, h * T:(h + 1) * T], qpT[h])
            wkcT_sb = res.tile([128, HPC, 4, NOPE], BF16, tag="wkct")
            for h in range(HPC):
                for c in range(4):
                    nc.scalar.dma_start(wkcT_sb[:, h, c, :], wkcT[h, c])
            wv2_sb = res.tile([128, 4, HPC * VDIM], BF16, tag="wv2")
            for c in range(4):
                nc.scalar.dma_start(wv2_sb[:, c, :], wv2[c])

            ones_col = res.tile([128, 1], FP32, tag="ones_col")
            nc.vector.memset(ones_col[:], 1.0)
            ones_f32 = res.tile([1, 1], FP32, tag="ones_f32")
            nc.vector.memset(ones_f32[:], 1.0)
            ident = res.tile([128, 128], FP32, tag="ident")
            make_identity(nc, ident[:])

            # ---------------- kv stream (512-col blocks; kvr on gpsimd queue
            # so the sync queue is pure kvT and k/v-gen never waits)
            kvT_sb = [
                res.tile([128, S], BF16, tag=f"kvt{c}", name=f"kvt{c}")
                for c in range(4)
            ]
            kvr_sb = res.tile([128, S], BF16, tag="kvr")

            def load_kv_block(b):
                sl = slice(b * 512, (b + 1) * 512)
                for c in range(4):
                    nc.sync.dma_start(kvT_sb[c][:, sl], kvT[c * 128:(c + 1) * 128, sl])
                nc.gpsimd.dma_start(kvr_sb[:, sl], kvr[:, sl])

            for b in range(16):
                load_kv_block(b)

            # ---------------- k/v generation (un-absorbed projections)
            kT_sb = [
                res.tile([NOPE, S], BF16, tag=f"kt{h}", name=f"kt{h}")
                for h in range(HPC)
            ]
            v_sb = res.tile([128, HPC * S], BF16, tag="vsb")

            def kgen(h, sb):
                sl = slice(sb * 512, (sb + 1) * 512)
                kps = ps_sc.tile([128, 1024], FP32, tag="sc", name="kps")
                for c in range(4):
                    nc.tensor.matmul(
                        kps[:, 0:512],
                        wkcT_sb[:, h, c, :],
                        kvT_sb[c][:, sl],
                        kind="kgen",
                        start=(c == 0),
                        stop=(c == 3),
                    )
                nc.vector.tensor_copy(kT_sb[h][:, sl], kps[:, 0:512])

            def vgen(sb):
                for st in range(4):
                    s = sb * 4 + st
                    vps = ps_sc.tile([128, 1024], FP32, tag="sc", name="vps")
                    for c in range(4):
                        nc.tensor.matmul(
                            vps[:, 0:HPC * VDIM],
                            kvT_sb[c][:, s * 128:(s + 1) * 128],
                            wv2_sb[:, c, :],
                            kind="vgen",
                            start=(c == 0),
                            stop=(c == 3),
                        )
                    nc.vector.tensor_copy(
                        v_sb[:, s * (HPC * VDIM):(s + 1) * (HPC * VDIM)],
                        vps[:, 0:HPC * VDIM],
                    )

            def gen_block(sb):
                kgen(0, sb)
                vgen(sb)

            gen_block(0)
            gen_block(1)

            # ---------------- main phases: (head, t-block)
            # gen_hook(ph, sp) interleaves leftover k/v-gen into early phases
            def gen_hook(ph, sp):
                if ph == 0 and sp + 2 < 16:
                    gen_block(sp + 2)
                elif ph == 1:
                    kgen(1, sp)

            epi_pending = [None]

            def run_phase(ph):
                h, tb = divmod(ph, NTB)
                tq = slice(h * T + tb * 512, h * T + (tb + 1) * 512)
                attn_ps = ps_attn.tile([128, 512], FP32, tag="attn")
                acc = smsb.tile([128, 512], FP32, tag="acc")
                acc2 = smsb.tile([128, 512], FP32, tag="acc2")
                pending = []  # (ss, pt2s) two groups back: PV runs well
                # behind exp so ACT latency never stalls PE

                def emit_pv(pv_ss, pv_pt2s):
                    for i, s in enumerate(pv_ss):
                        nc.tensor.matmul(
                            attn_ps[:],
                            v_sb[:, s * 256 + h * VDIM:
                                 s * 256 + h * VDIM + VDIM],
                            pv_pt2s[i // 2][:, (i % 2) * 512:(i % 2) * 512 + 512],
                            kind="pv",
                            start=(s == 0),
                            stop=(s == NST - 1),
                        )

                for sp in range(NST // 4):
                    if sp == 2 and epi_pending[0] is not None:
                        epi_pending[0]()
                        epi_pending[0] = None
                    gen_hook(ph, sp)
                    ss = tuple(4 * sp + k for k in range(4))
                    # two [128,1024] score tiles per group; each holds a pair
                    # of s-tiles so one ACT exp covers both halves
                    scs = [ps_sc.tile([128, 1024], FP32, tag="sc",
                                      name="sc_ps") for _ in range(2)]
                    for i, s in enumerate(ss):
                        lo = (i % 2) * ROPE
                        nc.tensor.matmul(
                            scs[i // 2][:, (i % 2) * 512:(i % 2) * 512 + 512],
                            kvr_sb[lo:lo + ROPE, s * 128:(s + 1) * 128],
                            qpT_sb[lo:lo + ROPE, tq],
                            kind="rope",
                            start=True,
                            stop=False,
                            tile_position=(lo, 0),
                        )
                    pt2s = []
                    for i, s in enumerate(ss):
                        nc.tensor.matmul(
                            scs[i // 2][:, (i % 2) * 512:(i % 2) * 512 + 512],
                            kT_sb[h][:, s * 128:(s + 1) * 128],
                            qnT_sb[:, tq],
                            kind="nope",
                            start=False,
                            stop=True,
                        )
                        if i % 2 == 1:
                            pt2 = ptp.tile([128, 1024], BF16, tag="pt",
                                           name="pt")
                            nc.scalar.activation(pt2[:], scs[i // 2][:], Exp,
                                                 scale=SCALING)
                            # denominator partials alternate DVE / GpSimd
                            if i == 1:
                                eng, dst = nc.vector, acc
                            else:
                                eng, dst = nc.gpsimd, acc2
                            first = (s <= 3)
                            for half in range(2):
                                psl = pt2[:, half * 512:half * 512 + 512]
                                if first and half == 0:
                                    eng.tensor_copy(dst[:], psl)
                                else:
                                    eng.tensor_add(dst[:], dst[:], psl)
                            pt2s.append(pt2)

                    pending.append((ss, pt2s))
                    if len(pending) > 2:
                        emit_pv(*pending.pop(0))
                for p in pending:
                    emit_pv(*p)

                # drain attn accumulator now (frees the PSUM bank for the
                # next phase); the rest of the epilogue is deferred into the
                # next phase's early groups
                attn_sb = attnsb.tile([128, 512], FP32, tag="attn")
                nc.vector.tensor_copy(attn_sb[:], attn_ps[:])

                def epilogue():
                    nc.vector.tensor_add(acc[:], acc[:], acc2[:])
                    den_ps = ps_out.tile([1, 512], FP32, tag="out",
                                         name="den_ps")
                    nc.tensor.matmul(den_ps[:], ones_col[:], acc[:])
                    den_sb = smsb.tile([1, 512], FP32, tag="den")
                    nc.vector.tensor_copy(den_sb[:], den_ps[:])
                    dT_ps = ps_out.tile([128, 4], FP32, tag="out",
                                        name="dT_ps")
                    for j in range(4):
                        nc.tensor.matmul(
                            dT_ps[:, j:j + 1],
                            den_sb[0:1, j * 128:(j + 1) * 128],
                            ones_f32[0:1, 0:1],
                        )
                    rc = smsb.tile([128, 4], FP32, tag="recip")
                    nc.vector.reciprocal(rc[:], dT_ps[:])

                    # output: PE-transpose attnT [128 v, 512 t] into [t, v]
                    # tiles, scale by 1/denom, DMA out
                    for j in range(4):
                        tp = ps_out.tile([128, 128], FP32, tag="out",
                                         name="tp")
                        nc.tensor.transpose(
                            tp[:], attn_sb[:, j * 128:(j + 1) * 128], ident[:]
                        )
                        ot = outsb.tile([128, VDIM], FP32, tag="out")
                        nc.vector.tensor_scalar_mul(ot[:], tp[:],
                                                    rc[:, j:j + 1])
                        nc.sync.dma_start(
                            out[tb * 512 + j * 128:tb * 512 + (j + 1) * 128,
                                h * VDIM:(h + 1) * VDIM],
                            ot[:],
                        )

                epi_pending[0] = epilogue

            for ph in range(HPC * NTB):
                run_phase(ph)
            epi_pending[0]()
    return nc


_PROGRAM = None


def _get_program():
    global _PROGRAM
    if _PROGRAM is None:
        _PROGRAM = _build_program()
    return _PROGRAM


# ---------------------------------------------------------------- host side
last_results = None  # BassKernelResults of the most recent run (for test.py)


def kernel(q, kv_cache, w_kc, w_vc):
    q = np.asarray(q, dtype=np.float32)
    kv_cache = np.asarray(kv_cache, dtype=np.float32)
    w_kc = np.asarray(w_kc, dtype=np.float32)
    w_vc = np.asarray(w_vc, dtype=np.float32)

    kvT_full = np.ascontiguousarray(kv_cache.T).astype(NPBF)       # [576, S]
    kvT_np = kvT_full[:LORA]                                        # [512, S]
    kvr_np = np.concatenate([kvT_full[LORA:], kvT_full[LORA:]], 0)  # [128, S] rope x2

    in_maps = []
    for core in range(N_CORES):
        hs = [core * HPC + i for i in range(HPC)]
        qnT_np = np.stack(
            [np.ascontiguousarray(q[:, h, :NOPE].T) for h in hs]
        ).astype(NPBF)                                              # [HPC,128,T]
        qpT_np = np.stack(
            [np.ascontiguousarray(q[:, h, NOPE:].T) for h in hs]
        ).astype(NPBF)                                              # [HPC,64,T]
        # wkcT[h, c, l, n] = w_kc[h, n, c*128+l]  (lhsT for k-gen)
        wkcT_np = np.ascontiguousarray(
            w_kc[hs].transpose(0, 2, 1).reshape(HPC, 4, 128, NOPE)
        ).astype(NPBF)
        # wv2[c, l, h*128+v] = w_vc[h, c*128+l, v]  (rhs for v-gen, both heads)
        wv2_np = np.ascontiguousarray(
            w_vc[hs].transpose(1, 0, 2).reshape(4, 128, HPC * VDIM)
        ).astype(NPBF)
        in_maps.append(
            {
                "qnT": qnT_np,
                "qpT": qpT_np,
                "kvT": kvT_np,
                "kvr": kvr_np,
                "wkcT": wkcT_np,
                "wv2": wv2_np,
            }
        )

    nc = _get_program()
    trace = bool(int(os.environ.get("KERNEL_TRACE", "0")))
    trace_cores = None
    if trace and os.environ.get("KERNEL_TRACE_CORES"):
        trace_cores = [
            int(x) for x in os.environ["KERNEL_TRACE_CORES"].split(",")
        ]
    res = run_bass_kernel_spmd(
        nc,
        in_maps,
        core_ids=list(range(N_CORES)),
        trace=trace,
        trace_cores=trace_cores,
    )
    global last_results
    last_results = res

    full = np.concatenate([res.results[c]["out"] for c in range(N_CORES)], axis=1)
    return np.ascontiguousarray(full.astype(np.float32))


# revision 10
# speedup vs baseline: 1.1454x; 1.1454x over previous
"""DeepseekV2 MLA (chunked-softmax MQA) on 8 trn2 NeuronCores.

Sharding: tensor-parallel over heads (16 heads / 8 cores = 2 heads per core);
the 576-wide latent KV cache is replicated per core. Each core computes its two
heads' full attention output [1024, 256]; the host concatenates along the
feature axis. All matmuls run in bf16 with fp32 PSUM accumulation.

Instead of the weight-absorbed form (score K=576, PV over the 512-wide
latent), each core un-absorbs the projections for its two heads up front:
  kT_h = w_kc[h] @ kv_loraT                 (PE, [128 nope, S])
  v    = kv_loraT.T @ [w_vc[h0]|w_vc[h1]]   ([128 s, 256] tiles)
so the inner loops shrink to
  scoresT = kT_h.T @ q_nopeT + ropeT.T @ q_peT   (K=128 + K=64 paired)
  pT      = exp(scale * scoresT)                  (ACT, PSUM->SBUF bf16)
  attnT  += v_tile.T @ pT                         (PE accum, [128 v, 512 t])
  out     = attnT.T via PE transpose, * 1/denom, DMA out.

Pipeline balancing (phase B is ACT/DVE-gated otherwise):
  - score PSUM tiles are allocated as [128, 1024] pairs so one ACT exp
    covers two s-tiles (amortizes ACT instruction overhead)
  - softmax denominator partial sums alternate DVE / GpSimd
  - PV matmuls lag two s-tile groups behind so exp latency never stalls PE
  - k-gen (head 0) + v-gen interleave with the first phase's groups, and
    k-gen (head 1) with the second phase's, filling PE while ACT is the
    per-group rate limiter
  - each phase's epilogue (denominator transpose, output transposes, DMA)
    is deferred into the next phase's early groups
"""

import os
import sys

import numpy as np
import ml_dtypes

for _p in ("/opt/trn_rl_repo",):
    if os.path.isdir(_p) and _p not in sys.path:
        sys.path.append(_p)

import concourse.bass as bass
import concourse.mybir as mybir
import concourse.tile as tile
from concourse.bass_utils import run_bass_kernel_spmd
from concourse.masks import make_identity
from concourse.vector_clock import ScopedClock, VectorClock

# ---------------------------------------------------------------- constants
NOPE, ROPE, LORA, VDIM = 128, 64, 512, 128
T, H, S = 1024, 16, 8192
D = LORA + ROPE            # 576 latent dim
SCALING = (NOPE + ROPE) ** -0.5
N_CORES = 8
HPC = H // N_CORES         # heads per core
NST = S // 128             # 64 s-tiles
NTB = T // 512             # 2 t-blocks
BF16 = mybir.dt.bfloat16
FP32 = mybir.dt.float32
NPBF = ml_dtypes.bfloat16


# ------------------------------------------------- walrus drain workaround
def _patch_tile_drain():
    """The neuronxcc walrus in this container rejects DRAIN instructions
    carrying more than ~2 sync waits ("Too many sync wait commands").
    Split the TileContext exit drain into one drain per processor tick;
    the waits execute sequentially on SP before the all-engine barrier,
    preserving the original semantics."""
    if getattr(tile.TileContext, "_drain_split_patched", False):
        return

    def _drain_and_barrier_split(self, tick_clock, wait_clock):
        gcv = tick_clock.global_clock
        n = len(gcv)
        for proc in range(n):
            t = gcv[proc]
            if t <= 0:
                continue
            vc = VectorClock([0] * n)
            vc.require_at_least(proc, t)
            d = self.nc.sync.drain()
            wait_clock.add_sem_waits(d.ins, ScopedClock({None: vc}))
        self.nc.all_engine_barrier()
        assert self.sems is not None
        popped = self.nc._tile_sem_poison_stack.pop()
        assert popped is self._sem_poison
        self.nc.clear_and_free_semaphores(list(self.sems.allocated().values()))
        self.nc.all_engine_barrier()

    tile.TileContext._drain_and_barrier = _drain_and_barrier_split

    # Same walrus limitation for regular instructions: peel all but the last
    # sync wait off onto same-engine NOPs inserted immediately before the
    # instruction. The engine executes its queue in order, so waiting on the
    # NOPs first is equivalent to one multi-wait instruction.
    orig_add = tile.TileContext._add_instruction

    def _add_instruction_split_waits(self, inst):
        si = inst.sync_info
        if si is not None:
            waits = si.on_wait
            if waits and len(waits) > 1:
                for w in waits[:-1]:
                    nop = mybir.InstNoOp(
                        name=self.nc.get_next_instruction_name(), ins=[], outs=[]
                    )
                    nop.engine = inst.engine
                    nop.sync_info = mybir.SyncInfo(on_wait=[w], on_update=[])
                    orig_add(self, nop)
                inst.sync_info = mybir.SyncInfo(
                    on_wait=[waits[-1]], on_update=si.on_update
                )
        orig_add(self, inst)

    tile.TileContext._add_instruction = _add_instruction_split_waits
    tile.TileContext._drain_split_patched = True


# ------------------------------------------------------------ bass program
MM_KINDS = {}


def _build_program():
    _patch_tile_drain()
    nc = bass.Bass()
    _orig_mm = nc.tensor.matmul

    def _mm_logged(out, lhsT, rhs, kind="?", **kw):
        inst = _orig_mm(out, lhsT, rhs, **kw)
        MM_KINDS[inst.ins.name] = kind
        return inst

    nc.tensor.matmul = _mm_logged
    qnT = nc.declare_dram_parameter("qnT", [HPC, NOPE, T], BF16, isOutput=False)
    qpT = nc.declare_dram_parameter("qpT", [HPC, ROPE, T], BF16, isOutput=False)
    kvT = nc.declare_dram_parameter("kvT", [LORA, S], BF16, isOutput=False)
    kvr = nc.declare_dram_parameter("kvr", [128, S], BF16, isOutput=False)
    wkcT = nc.declare_dram_parameter("wkcT", [HPC, 4, 128, NOPE], BF16, isOutput=False)
    wv2 = nc.declare_dram_parameter("wv2", [4, 128, HPC * VDIM], BF16, isOutput=False)
    out = nc.declare_dram_parameter("out", [T, HPC * VDIM], FP32, isOutput=True)

    Exp = mybir.ActivationFunctionType.Exp

    with tile.TileContext(nc) as tc:
        with (
            tc.tile_pool(name="res", bufs=1) as res,
            tc.tile_pool(name="ptp", bufs=6) as ptp,
            tc.tile_pool(name="attnsb", bufs=2) as attnsb,
            tc.tile_pool(name="smsb", bufs=3) as smsb,
            tc.tile_pool(name="outsb", bufs=3) as outsb,
            tc.tile_pool(name="ps_sc", bufs=3, space="PSUM") as ps_sc,
            tc.tile_pool(name="ps_attn", bufs=1, space="PSUM") as ps_attn,
            tc.tile_pool(name="ps_out", bufs=1, space="PSUM") as ps_out,
        ):
            # PE warmup first: ~5us of matmuls on local data so HAM
            # un-throttles and the first real matmuls run at full clock
            warm = res.tile([128, 512], BF16, tag="warm")
            nc.vector.memset(warm[:], 0.0)
            wu_ps = ps_sc.tile([128, 1024], FP32, tag="sc", name="wu_ps")
            for _w in range(20):
                nc.tensor.matmul(wu_ps[:, 0:512], warm[:, 0:128], warm[:],
                                 kind="warm")

            # ---------------- resident loads (gen weights first: k/v-gen
            # needs them within ~8us; q tensors aren't read until phase B)
            wkcT_sb = res.tile([128, HPC, 4, NOPE], BF16, tag="wkct")
            for h in range(HPC):
                for c in range(4):
                    nc.scalar.dma_start(wkcT_sb[:, h, c, :], wkcT[h, c])
            wv2_sb = res.tile([128, 4, HPC * VDIM], BF16, tag="wv2")
            for c in range(4):
                nc.scalar.dma_start(wv2_sb[:, c, :], wv2[c])
            qnT_sb = res.tile([NOPE, HPC * T], BF16, tag="qnt")
            qpT_sb = res.tile([128, HPC * T], BF16, tag="qpt")
            for h in range(HPC):
                nc.scalar.dma_start(qnT_sb[:, h * T:(h + 1) * T], qnT[h])
                # rope rows duplicated to partitions 64:128 so pairs of K=64
                # rope matmuls run concurrently in disjoint PE row-groups
                nc.scalar.dma_start(qpT_sb[0:ROPE, h * T:(h + 1) * T], qpT[h])
                nc.scalar.dma_start(qpT_sb[ROPE:128, h * T:(h + 1) * T], qpT[h])

            ones_col = res.tile([128, 1], BF16, tag="ones_col")
            nc.vector.memset(ones_col[:], 1.0)
            ones_f32 = res.tile([1, 1], FP32, tag="ones_f32")
            nc.vector.memset(ones_f32[:], 1.0)
            ident = res.tile([128, 128], FP32, tag="ident")
            make_identity(nc, ident[:])

            # ---------------- kv stream (512-col blocks; kvr on gpsimd queue
            # so the sync queue is pure kvT and k/v-gen never waits)
            kvT_sb = [
                res.tile([128, S], BF16, tag=f"kvt{c}", name=f"kvt{c}")
                for c in range(4)
            ]
            kvr_sb = res.tile([128, S], BF16, tag="kvr")

            def load_kv_block(b):
                sl = slice(b * 512, (b + 1) * 512)
                for c in range(4):
                    nc.sync.dma_start(kvT_sb[c][:, sl], kvT[c * 128:(c + 1) * 128, sl])
                nc.gpsimd.dma_start(kvr_sb[:, sl], kvr[:, sl])

            for b in range(16):
                load_kv_block(b)

            # ---------------- k/v generation (un-absorbed projections)
            kT_sb = [
                res.tile([NOPE, S], BF16, tag=f"kt{h}", name=f"kt{h}")
                for h in range(HPC)
            ]
            v_sb = res.tile([128, HPC * S], BF16, tag="vsb")

            def kgen(h, sb):
                sl = slice(sb * 512, (sb + 1) * 512)
                kps = ps_sc.tile([128, 1024], FP32, tag="sc", name="kps")
                for c in range(4):
                    nc.tensor.matmul(
                        kps[:, 0:512],
                        wkcT_sb[:, h, c, :],
                        kvT_sb[c][:, sl],
                        kind="kgen",
                        start=(c == 0),
                        stop=(c == 3),
                    )
                nc.vector.tensor_copy(kT_sb[h][:, sl], kps[:, 0:512])

            def vgen(sb):
                for st in range(4):
                    s = sb * 4 + st
                    vps = ps_sc.tile([128, 1024], FP32, tag="sc", name="vps")
                    for c in range(4):
                        nc.tensor.matmul(
                            vps[:, 0:HPC * VDIM],
                            kvT_sb[c][:, s * 128:(s + 1) * 128],
                            wv2_sb[:, c, :],
                            kind="vgen",
                            start=(c == 0),
                            stop=(c == 3),
                        )
                    nc.vector.tensor_copy(
                        v_sb[:, s * (HPC * VDIM):(s + 1) * (HPC * VDIM)],
                        vps[:, 0:HPC * VDIM],
                    )

            def gen_block(sb):
                kgen(0, sb)
                vgen(sb)

            gen_block(0)
            gen_block(1)

            # ---------------- main phases: (head, t-block)
            # gen_hook(ph, sp) interleaves leftover k/v-gen into early phases
            def gen_hook(ph, sp):
                if ph == 0 and sp + 2 < 16:
                    gen_block(sp + 2)
                elif ph == 1:
                    kgen(1, sp)

            epi_pending = [None]

            def run_phase(ph):
                h, tb = divmod(ph, NTB)
                tq = slice(h * T + tb * 512, h * T + (tb + 1) * 512)
                attn_ps = ps_attn.tile([128, 512], FP32, tag="attn")
                # softmax denominator partials: bf16 accumulators (all-2-byte
                # operands unlock the DVE 2x perf mode), one [128,1024] add
                # per exp pair, alternating DVE / GpSimd
                accv = smsb.tile([128, 1024], BF16, tag="accv")
                accg = smsb.tile([128, 1024], BF16, tag="accg")
                pending = []  # (ss, pt2s) two groups back: PV runs well
                # behind exp so ACT latency never stalls PE

                def emit_pv(pv_ss, pv_pt2s):
                    for i, s in enumerate(pv_ss):
                        nc.tensor.matmul(
                            attn_ps[:],
                            v_sb[:, s * 256 + h * VDIM:
                                 s * 256 + h * VDIM + VDIM],
                            pv_pt2s[i // 2][:, (i % 2) * 512:(i % 2) * 512 + 512],
                            kind="pv",
                            start=(s == 0),
                            stop=(s == NST - 1),
                        )

                for sp in range(NST // 4):
                    if sp == 4 and epi_pending[0] is not None:
                        epi_pending[0]()
                        epi_pending[0] = None
                    gen_hook(ph, sp)
                    ss = tuple(4 * sp + k for k in range(4))
                    # two [128,1024] score tiles per group; each holds a pair
                    # of s-tiles so one ACT exp covers both halves
                    scs = [ps_sc.tile([128, 1024], FP32, tag="sc",
                                      name="sc_ps") for _ in range(2)]
                    for i, s in enumerate(ss):
                        lo = (i % 2) * ROPE
                        nc.tensor.matmul(
                            scs[i // 2][:, (i % 2) * 512:(i % 2) * 512 + 512],
                            kvr_sb[lo:lo + ROPE, s * 128:(s + 1) * 128],
                            qpT_sb[lo:lo + ROPE, tq],
                            kind="rope",
                            start=True,
                            stop=False,
                            tile_position=(lo, 0),
                        )
                    pt2s = []
                    for i, s in enumerate(ss):
                        nc.tensor.matmul(
                            scs[i // 2][:, (i % 2) * 512:(i % 2) * 512 + 512],
                            kT_sb[h][:, s * 128:(s + 1) * 128],
                            qnT_sb[:, tq],
                            kind="nope",
                            start=False,
                            stop=True,
                        )
                        if i % 2 == 1:
                            pt2 = ptp.tile([128, 1024], BF16, tag="pt",
                                           name="pt")
                            nc.scalar.activation(pt2[:], scs[i // 2][:], Exp,
                                                 scale=SCALING)
                            if i == 1:
                                eng, dst = nc.vector, accv
                            else:
                                eng, dst = nc.gpsimd, accg
                            if sp == 0:
                                eng.tensor_copy(dst[:], pt2[:])
                            else:
                                eng.tensor_add(dst[:], dst[:], pt2[:])
                            pt2s.append(pt2)

                    pending.append((ss, pt2s))
                    if len(pending) > 2:
                        emit_pv(*pending.pop(0))
                for p in pending:
                    emit_pv(*p)

                # drain attn accumulator now (frees the PSUM bank for the
                # next phase); the rest of the epilogue is deferred into the
                # next phase's early groups
                attn_sb = attnsb.tile([128, 512], FP32, tag="attn")
                nc.vector.tensor_copy(attn_sb[:], attn_ps[:])

                def epilogue():
                    fold = smsb.tile([128, 1024], BF16, tag="fold")
                    nc.vector.tensor_add(fold[:], accv[:], accg[:])
                    den_ps = ps_out.tile([1, 512], FP32, tag="out",
                                         name="den_ps")
                    for half in range(2):
                        nc.tensor.matmul(
                            den_ps[:],
                            ones_col[:],
                            fold[:, half * 512:half * 512 + 512],
                            start=(half == 0),
                            stop=(half == 1),
                        )
                    den_sb = smsb.tile([1, 512], FP32, tag="den")
                    nc.vector.tensor_copy(den_sb[:], den_ps[:])
                    dT_ps = ps_out.tile([128, 4], FP32, tag="out",
                                        name="dT_ps")
                    for j in range(4):
                        nc.tensor.matmul(
                            dT_ps[:, j:j + 1],
                            den_sb[0:1, j * 128:(j + 1) * 128],
                            ones_f32[0:1, 0:1],
                        )
                    rc = smsb.tile([128, 4], FP32, tag="recip")
                    nc.vector.reciprocal(rc[:], dT_ps[:])

                    # output: PE-transpose attnT [128 v, 512 t] into [t, v]
                    # tiles, scale by 1/denom, DMA out
                    for j in range(4):
                        tp = ps_out.tile([128, 128], FP32, tag="out",
                                         name="tp")
                        nc.tensor.transpose(
                            tp[:], attn_sb[:, j * 128:(j + 1) * 128], ident[:]
                        )
                        ot = outsb.tile([128, VDIM], FP32, tag="out")
                        nc.vector.tensor_scalar_mul(ot[:], tp[:],
                                                    rc[:, j:j + 1])
                        nc.sync.dma_start(
                            out[tb * 512 + j * 128:tb * 512 + (j + 1) * 128,
                                h * VDIM:(h + 1) * VDIM],
                            ot[:],
                        )

                epi_pending[0] = epilogue

            for ph in range(HPC * NTB):
                run_phase(ph)
            epi_pending[0]()
    return nc


_PROGRAM = None


def _get_program():
    global _PROGRAM
    if _PROGRAM is None:
        _PROGRAM = _build_program()
    return _PROGRAM


# ---------------------------------------------------------------- host side
last_results = None  # BassKernelResults of the most recent run (for test.py)


def kernel(q, kv_cache, w_kc, w_vc):
    q = np.asarray(q, dtype=np.float32)
    kv_cache = np.asarray(kv_cache, dtype=np.float32)
    w_kc = np.asarray(w_kc, dtype=np.float32)
    w_vc = np.asarray(w_vc, dtype=np.float32)

    kvT_full = np.ascontiguousarray(kv_cache.T).astype(NPBF)       # [576, S]
    kvT_np = kvT_full[:LORA]                                        # [512, S]
    kvr_np = np.concatenate([kvT_full[LORA:], kvT_full[LORA:]], 0)  # [128, S] rope x2

    in_maps = []
    for core in range(N_CORES):
        hs = [core * HPC + i for i in range(HPC)]
        qnT_np = np.stack(
            [np.ascontiguousarray(q[:, h, :NOPE].T) for h in hs]
        ).astype(NPBF)                                              # [HPC,128,T]
        qpT_np = np.stack(
            [np.ascontiguousarray(q[:, h, NOPE:].T) for h in hs]
        ).astype(NPBF)                                              # [HPC,64,T]
        # wkcT[h, c, l, n] = w_kc[h, n, c*128+l]  (lhsT for k-gen)
        wkcT_np = np.ascontiguousarray(
            w_kc[hs].transpose(0, 2, 1).reshape(HPC, 4, 128, NOPE)
        ).astype(NPBF)
        # wv2[c, l, h*128+v] = w_vc[h, c*128+l, v]  (rhs for v-gen, both heads)
        wv2_np = np.ascontiguousarray(
            w_vc[hs].transpose(1, 0, 2).reshape(4, 128, HPC * VDIM)
        ).astype(NPBF)
        in_maps.append(
            {
                "qnT": qnT_np,
                "qpT": qpT_np,
                "kvT": kvT_np,
                "kvr": kvr_np,
                "wkcT": wkcT_np,
                "wv2": wv2_np,
            }
        )

    nc = _get_program()
    trace = bool(int(os.environ.get("KERNEL_TRACE", "0")))
    trace_cores = None
    if trace and os.environ.get("KERNEL_TRACE_CORES"):
        trace_cores = [
            int(x) for x in os.environ["KERNEL_TRACE_CORES"].split(",")
        ]
    res = run_bass_kernel_spmd(
        nc,
        in_maps,
        core_ids=list(range(N_CORES)),
        trace=trace,
        trace_cores=trace_cores,
    )
    global last_results
    last_results = res

    full = np.concatenate([res.results[c]["out"] for c in range(N_CORES)], axis=1)
    return np.ascontiguousarray(full.astype(np.float32))
